# revision 1
# baseline (speedup 1.0000x reference)
"""Trainium2 Bass kernel for nn_Attention_30760555774660 (stacked attention VQA net).

Sharding: data-parallel over batch, 256 -> 8 cores x 32. Weights replicated.

Per-core plan (B=32 local batch, S=196, D=1024, A=512, O=3000):
  - img_b [196,1024] DMA'd once, PE-transposed (identity matmuls) into
    imgT_b [128, 8, 196] (d-on-partitions) for the two projections.
  - Projections img_b @ W_ia{1,2} run as float32r (full-rate fp32) matmuls,
    N=512, accumulating 8 K-chunks in PSUM.
  - The broadcast add of the q-projection row is folded into the same PSUM
    accumulation with a one-hot selector matmul (K=32).
  - tanh on ScalarE; logits via DVE tensor_tensor_reduce against
    partition-broadcast Wp; softmax batched per group of 4 batch elems
    ([4,196] rows after a PE transpose of the logit columns).
  - vI = pi @ img via matmul with pi column stationary, natural img moving.
  - u1/u2 kept transposed (u1T/u2T [128, 8, 32]) so the q-projection of
    block 2 and the final FC need no extra transposes.
  - Final FC streams W_fc [128,500] tiles against stationary u2T.
"""

import os
import sys

import numpy as np

if "/opt/trn_rl_repo" not in sys.path:
    sys.path.insert(0, "/opt/trn_rl_repo")

B_FULL = 256
N_CORES = 8
B = B_FULL // N_CORES  # 32
S = 196
D = 1024
A = 512
O = 3000
G = 4  # softmax group size
NG = B // G
DC = D // 128  # 8 d-chunks
S_CHUNKS = [(0, 128), (128, 68)]
OC = 6
ON = O // OC  # 500

_nc_cache = None


def _build_nc():
    import concourse.bacc as bacc
    import concourse.tile as tile
    from concourse import mybir

    f32 = mybir.dt.float32
    f32r = mybir.dt.float32r
    Tanh = mybir.ActivationFunctionType.Tanh
    Exp = mybir.ActivationFunctionType.Exp
    mult = mybir.AluOpType.mult
    add = mybir.AluOpType.add

    nc = bacc.Bacc("TRN2", target_bir_lowering=False)

    img_h = nc.dram_tensor("img", [B, S, D], f32r, kind="ExternalInput")
    ques_h = nc.dram_tensor("ques", [B, D], f32, kind="ExternalInput")
    wia1_h = nc.dram_tensor("W_ia1", [D, A], f32r, kind="ExternalInput")
    wqa1_h = nc.dram_tensor("W_qa1", [D, A], f32r, kind="ExternalInput")
    bqa1_h = nc.dram_tensor("b_qa1", [A], f32, kind="ExternalInput")
    wp1_h = nc.dram_tensor("Wp1", [A], f32, kind="ExternalInput")
    wia2_h = nc.dram_tensor("W_ia2", [D, A], f32r, kind="ExternalInput")
    wqa2_h = nc.dram_tensor("W_qa2", [D, A], f32r, kind="ExternalInput")
    bqa2_h = nc.dram_tensor("b_qa2", [A], f32, kind="ExternalInput")
    wp2_h = nc.dram_tensor("Wp2", [A], f32, kind="ExternalInput")
    wfc_h = nc.dram_tensor("W_fc", [D, O], f32r, kind="ExternalInput")
    bfc_h = nc.dram_tensor("b_fc", [O], f32, kind="ExternalInput")
    oneh_h = nc.dram_tensor("ONEHOTS", [B, B, 128], f32r, kind="ExternalInput")
    ident_h = nc.dram_tensor("IDENT", [128, 128], f32r, kind="ExternalInput")
    identf_h = nc.dram_tensor("IDENTF", [128, 128], f32, kind="ExternalInput")
    score_h = nc.dram_tensor("score", [B, O], f32, kind="ExternalOutput")

    import bass_rust  # noqa: F401
    import concourse.bass as bass  # noqa: F401

    def bcast_ap(h, n_part, free_n):
        # partition-stride-0 broadcast read of a 1-D dram tensor
        ap = h[:]
        return bass.AP(tensor=ap.tensor, offset=ap.offset, ap=[[0, n_part]] + ap.ap)

    with tile.TileContext(nc) as tc:
        with (
            tc.tile_pool(name="const", bufs=1) as const,
            tc.tile_pool(name="imgn", bufs=5) as imgn_p,
            tc.tile_pool(name="imgt", bufs=3) as imgt_p,
            tc.tile_pool(name="p2sb", bufs=5) as p2sb_p,
            tc.tile_pool(name="work", bufs=2) as work,
            tc.tile_pool(name="work1", bufs=1) as work1,
            tc.tile_pool(name="wstream", bufs=3) as wstream,
            tc.tile_pool(name="pst", bufs=5, space="PSUM") as pst,
            tc.tile_pool(name="psp", bufs=3, space="PSUM") as psp,
        ):
            # ---------------- constants ----------------
            ident = const.tile([128, 128], f32r)
            nc.sync.dma_start(out=ident, in_=ident_h[:, :])
            identf = const.tile([128, 128], f32)
            nc.sync.dma_start(out=identf, in_=identf_h[:, :])
            oneh = const.tile([B, B, 128], f32r)
            nc.sync.dma_start(out=oneh, in_=oneh_h[:, :, :])
            wia1 = const.tile([128, DC, A], f32r)
            nc.sync.dma_start(out=wia1, in_=wia1_h[:, :].rearrange("(c p) a -> p c a", p=128))
            wia2 = const.tile([128, DC, A], f32r)
            nc.sync.dma_start(out=wia2, in_=wia2_h[:, :].rearrange("(c p) a -> p c a", p=128))
            wqa2 = const.tile([128, DC, A], f32r)
            nc.sync.dma_start(out=wqa2, in_=wqa2_h[:, :].rearrange("(c p) a -> p c a", p=128))
            bqa1b = const.tile([B, A], f32)
            nc.gpsimd.dma_start(out=bqa1b, in_=bcast_ap(bqa1_h, B, A))
            bqa2b = const.tile([B, A], f32)
            nc.gpsimd.dma_start(out=bqa2b, in_=bcast_ap(bqa2_h, B, A))
            wp1b = const.tile([128, A], f32)
            nc.gpsimd.dma_start(out=wp1b, in_=bcast_ap(wp1_h, 128, A))
            wp2b = const.tile([128, A], f32)
            nc.gpsimd.dma_start(out=wp2b, in_=bcast_ap(wp2_h, 128, A))
            quesA = const.tile([B, D], f32)
            nc.sync.dma_start(out=quesA, in_=ques_h[:, :])

            quesT = const.tile([128, DC, B], f32r)
            QP1 = const.tile([B, A], f32r)
            QP2 = const.tile([B, A], f32r)
            u1T = const.tile([128, DC, B], f32r)
            u2T = const.tile([128, DC, B], f32r)
            nc.vector.memset(QP2[:, :].bitcast(f32), 0.0)
            nc.vector.memset(u1T[:, :, :].bitcast(f32), 0.0)

            def r(ap):
                return ap

            # quesT[p, c, b] = ques[b, c*128+p]
            for c in range(DC):
                pt = pst.tile([128, B], f32, tag="tr")
                nc.tensor.transpose(pt, quesA[:, c * 128 : (c + 1) * 128], identf[0:B, 0:B])
                nc.vector.tensor_copy(quesT[:, c, :], pt)

            # QP1 = ques @ W_qa1 + b_qa1   [32, 512]
            qp_ps = psp.tile([B, A], f32, tag="pp")
            for c in range(DC):
                wq = wstream.tile([128, A], f32r, tag="ws")
                nc.sync.dma_start(out=wq, in_=wqa1_h[c * 128 : (c + 1) * 128, :])
                nc.tensor.matmul(qp_ps, r(quesT[:, c, :]), r(wq), start=(c == 0), stop=(c == DC - 1))
            nc.vector.tensor_add(QP1, qp_ps, bqa1b)

            imgN = {}
            imgT = {}
            p2sb = {}
            Lc1 = {}
            Lc2 = {}

            def load_and_proj(b):
                """DMA img_b, transpose, run both projections. Block-1 proj
                gets the QP1 broadcast folded in and goes through tanh+logits;
                block-2 proj parks in SBUF."""
                inb = imgn_p.tile([128, 2, D], f32r, tag="imgn")
                imgN[b] = inb
                nc.sync.dma_start(out=inb[:, 0, :], in_=img_h[b : b + 1, 0:128, :].rearrange("o s d -> (o s) d"))
                nc.sync.dma_start(out=inb[0:68, 1, :], in_=img_h[b : b + 1, 128:196, :].rearrange("o s d -> (o s) d"))
                itb = imgt_p.tile([128, DC, S], f32r, tag="imgt")
                imgT[b] = itb
                for c in range(DC):
                    pa = pst.tile([128, 128], f32, tag="tr")
                    nc.tensor.transpose(pa, inb[0:128, 0, c * 128 : (c + 1) * 128].bitcast(f32), identf)
                    nc.vector.tensor_copy(itb[:, c, 0:128], pa)
                    pb = pst.tile([128, 128], f32, tag="tr")
                    nc.tensor.transpose(pb[:, 0:68], inb[0:68, 1, c * 128 : (c + 1) * 128].bitcast(f32), identf[0:68, 0:68])
                    nc.vector.tensor_copy(itb[:, c, 128:196], pb[:, 0:68])
                # block-1 projection + QP1 broadcast + tanh + logits
                lc = work.tile([128, 2, G], f32, tag="lc1")
                if b % G == 0:
                    Lc1[b // G] = lc
                lc = Lc1[b // G]
                for si, (s0, sl) in enumerate(S_CHUNKS):
                    pp = psp.tile([128, A], f32, tag="pp")
                    for c in range(DC):
                        nc.tensor.matmul(
                            pp[0:sl, :], r(itb[0:128, c, s0 : s0 + sl]), r(wia1[:, c, :]),
                            start=(c == 0), stop=False,
                        )
                    nc.tensor.matmul(pp[0:sl, :], r(oneh[:, b, 0:sl]), r(QP1), start=False, stop=True)
                    ha = work.tile([128, A], f32, tag="ha")
                    nc.scalar.activation(ha[0:sl], pp[0:sl], Tanh)
                    prod = work.tile([128, A], f32, tag="prod")
                    nc.vector.tensor_mul(prod[0:sl], ha[0:sl], wp1b[0:sl])
                    nc.vector.tensor_reduce(
                        lc[0:sl, si, b % G : b % G + 1], prod[0:sl],
                        axis=mybir.AxisListType.X, op=add,
                    )
                # block-2 projection -> SBUF
                p2 = p2sb_p.tile([128, 2, A], f32, tag="p2sb")
                p2sb[b] = p2
                for si, (s0, sl) in enumerate(S_CHUNKS):
                    pp = psp.tile([128, A], f32, tag="pp")
                    for c in range(DC):
                        nc.tensor.matmul(
                            pp[0:sl, :], r(itb[0:128, c, s0 : s0 + sl]), r(wia2[:, c, :]),
                            start=(c == 0), stop=(c == DC - 1),
                        )
                    nc.vector.tensor_copy(p2[0:sl, si, :], pp[0:sl])

            def softmax_and_pi(lc, tagp):
                """logit columns [128, 2, G] -> PI [G, 196] softmax rows."""
                LT = work.tile([G, S], f32, tag="LT")
                pa = pst.tile([128, 128], f32, tag="tr")
                nc.tensor.transpose(pa[0:G, :], lc[:, 0, :], identf)
                nc.vector.tensor_copy(LT[:, 0:128], pa[0:G, :])
                pb = pst.tile([128, 128], f32, tag="tr")
                nc.tensor.transpose(pb[0:G, 0:68], lc[0:68, 1, :], identf[0:68, 0:68])
                nc.vector.tensor_copy(LT[:, 128:196], pb[0:G, 0:68])
                E = work.tile([G, S], f32, tag="E")
                Z = work.tile([G, 1], f32, tag="Z")
                nc.scalar.activation(E, LT, Exp, accum_out=Z)
                R = work.tile([G, 1], f32, tag="R")
                nc.vector.reciprocal(R, Z)
                PI = work.tile([G, S], f32, tag=tagp)
                nc.vector.tensor_scalar_mul(PI, E, R)
                return PI

            def weighted_sum_add(PI, g, other, out_tag, pool=None):
                """out[bb,:] = vI_bb + other[bb,:], via masked-column pi^T
                stationaries accumulating the whole group in one PSUM tile."""
                piTm = work.tile([128, 2, G, G], f32r, tag="piTm")
                for bb in range(G):
                    PIm = work.tile([G, S], f32, tag="PIm")
                    nc.vector.tensor_scalar_mul(PIm, PI, oneh[0:G, bb, 0:1].bitcast(f32))
                    pc = pst.tile([128, 128], f32, tag="tr")
                    nc.tensor.transpose(pc[:, 0:G], PIm[:, 0:128], identf[0:G, 0:G])
                    nc.vector.tensor_copy(piTm[:, 0, bb, :], pc[:, 0:G])
                    pd = pst.tile([128, 128], f32, tag="tr")
                    nc.tensor.transpose(pd[0:68, 0:G], PIm[:, 128:196], identf[0:G, 0:G])
                    nc.vector.tensor_copy(piTm[0:68, 1, bb, :], pd[0:68, 0:G])
                out = (pool or work).tile([G, D], f32, tag=out_tag)
                for h in range(2):
                    vp = psp.tile([G, A], f32, tag="pp")
                    k = 0
                    for bb in range(G):
                        inb = imgN[g * G + bb]
                        for si, (s0, sl) in enumerate(S_CHUNKS):
                            nc.tensor.matmul(
                                vp, r(piTm[0:sl, si, bb, :]), r(inb[0:sl, si, h * A : (h + 1) * A]),
                                start=(k == 0), stop=(k == 2 * G - 1),
                            )
                            k += 1
                    nc.vector.tensor_add(out[:, h * A : (h + 1) * A], vp, other[:, h * A : (h + 1) * A])
                return out

            NG_RUN = int(os.environ.get("NG_RUN", str(NG)))
            for g in range(NG_RUN):
                g4 = g * G
                for bb in range(G):
                    load_and_proj(g4 + bb)
                # ---- block 1 softmax / vI / u1 ----
                PI1 = softmax_and_pi(Lc1[g], "PI1")
                qn = work1.tile([G, D], f32, tag="qn")
                nc.sync.dma_start(out=qn, in_=ques_h[g4 : g4 + G, :])
                u1g = weighted_sum_add(PI1, g, qn, "u1g")
                for c in range(DC):
                    pt = pst.tile([128, 128], f32, tag="tr")
                    nc.tensor.transpose(pt[:, 0:G], u1g[:, c * 128 : (c + 1) * 128], identf[0:G, 0:G])
                    nc.vector.tensor_copy(u1T[:, c, g4 : g4 + G], pt[:, 0:G])
                # qp2 for this group (M=32, only rows g4:g4+G fresh)
                q2p = psp.tile([B, A], f32, tag="pp")
                for c in range(DC):
                    nc.tensor.matmul(q2p, r(u1T[:, c, :]), r(wqa2[:, c, :]), start=(c == 0), stop=(c == DC - 1))
                nc.vector.tensor_add(QP2, q2p, bqa2b)
                # ---- block 2 ----
                lc2 = work.tile([128, 2, G], f32, tag="lc2")
                Lc2[g] = lc2
                for bb in range(G):
                    b = g4 + bb
                    for si, (s0, sl) in enumerate(S_CHUNKS):
                        pp = psp.tile([128, A], f32, tag="pp")
                        nc.tensor.matmul(pp[0:sl, :], r(oneh[:, b, 0:sl]), r(QP2), start=True, stop=True)
                        ha2 = work.tile([128, A], f32, tag="ha")
                        nc.vector.tensor_add(ha2[0:sl], pp[0:sl], p2sb[b][0:sl, si, :])
                        nc.scalar.activation(ha2[0:sl], ha2[0:sl], Tanh)
                        prod = work.tile([128, A], f32, tag="prod")
                        nc.vector.tensor_mul(prod[0:sl], ha2[0:sl], wp2b[0:sl])
                        nc.vector.tensor_reduce(
                            lc2[0:sl, si, bb : bb + 1], prod[0:sl],
                            axis=mybir.AxisListType.X, op=add,
                        )
                PI2 = softmax_and_pi(lc2, "PI2")
                u2g = weighted_sum_add(PI2, g, u1g, "u2g", pool=work1)
                for c in range(DC):
                    pt = pst.tile([128, 128], f32, tag="tr")
                    nc.tensor.transpose(pt[:, 0:G], u2g[:, c * 128 : (c + 1) * 128], identf[0:G, 0:G])
                    nc.vector.tensor_copy(u2T[:, c, g4 : g4 + G], pt[:, 0:G])

            # ---------------- final FC ----------------
            for n in range(OC):
                fp = psp.tile([B, ON], f32, tag="pp")
                for c in range(DC):
                    wf = wstream.tile([128, ON], f32r, tag="ws")
                    nc.sync.dma_start(out=wf, in_=wfc_h[c * 128 : (c + 1) * 128, n * ON : (n + 1) * ON])
                    nc.tensor.matmul(fp, r(u2T[:, c, :]), r(wf), start=(c == 0), stop=(c == DC - 1))
                bf = work1.tile([B, ON], f32, tag="bf")
                nc.gpsimd.dma_start(
                    out=bf,
                    in_=_slice_bcast(bfc_h, B, n * ON, ON),
                )
                sc = work.tile([B, ON], f32, tag="sc")
                nc.vector.tensor_add(sc, fp, bf)
                nc.sync.dma_start(out=score_h[:, n * ON : (n + 1) * ON], in_=sc)

    nc.compile()
    return nc


def _slice_bcast(h, n_part, off, n):
    import concourse.bass as bass

    ap = h[off : off + n]
    return bass.AP(tensor=ap.tensor, offset=ap.offset, ap=[[0, n_part]] + ap.ap)


def _get_nc():
    global _nc_cache
    if _nc_cache is None:
        _nc_cache = _build_nc()
    return _nc_cache


def _make_in_maps(inputs):
    onehots = np.ascontiguousarray(
        np.repeat(np.eye(B, dtype=np.float32)[:, :, None], 128, axis=2)
    )
    ident = np.eye(128, dtype=np.float32)
    shared = {
        "W_ia1": np.ascontiguousarray(inputs["W_ia1"], np.float32),
        "W_qa1": np.ascontiguousarray(inputs["W_qa1"], np.float32),
        "b_qa1": np.ascontiguousarray(inputs["b_qa1"], np.float32),
        "Wp1": np.ascontiguousarray(inputs["Wp1"], np.float32),
        "W_ia2": np.ascontiguousarray(inputs["W_ia2"], np.float32),
        "W_qa2": np.ascontiguousarray(inputs["W_qa2"], np.float32),
        "b_qa2": np.ascontiguousarray(inputs["b_qa2"], np.float32),
        "Wp2": np.ascontiguousarray(inputs["Wp2"], np.float32),
        "W_fc": np.ascontiguousarray(inputs["W_fc"], np.float32),
        "b_fc": np.ascontiguousarray(inputs["b_fc"], np.float32),
        "ONEHOTS": onehots,
        "IDENT": ident,
        "IDENTF": ident,
    }
    in_maps = []
    for c in range(N_CORES):
        sl = slice(c * B, (c + 1) * B)
        m = dict(shared)
        m["img"] = np.ascontiguousarray(inputs["img_feat"][sl], np.float32)
        m["ques"] = np.ascontiguousarray(inputs["ques_feat"][sl], np.float32)
        in_maps.append(m)
    return in_maps


def kernel_run(inputs, trace=False):
    from concourse.bass_utils import run_bass_kernel_spmd

    nc = _get_nc()
    in_maps = _make_in_maps(inputs)
    res = run_bass_kernel_spmd(nc, in_maps, core_ids=list(range(N_CORES)), trace=trace)
    out = np.concatenate([r["score"] for r in res.results], axis=0)
    return out, res


def kernel(**inputs):
    out, _ = kernel_run(inputs)
    return out



# revision 17
# speedup vs baseline: 1.5240x; 1.5240x over previous
"""Trainium2 Bass kernel for nn_Attention_30760555774660 (stacked attention VQA).

Sharding: data-parallel over batch, 256 -> 8 cores x 32. Weights replicated.

Per-core structure (B=32, S=196, D=1024, A=512, O=3000), all matmuls bf16:
  - img is shipped bf16 and transposed during DRAM->SBUF load by the DMA
    XBAR (transpose=True), giving imgT [128d, 8c, 3136s] per 16-batch half.
    No PE transposes / PSUM copies for img at all.
  - Projection img @ W_ia runs s-flat: 25 chunks of [<=128 s, 512 a] PSUM,
    8 K-chunk matmuls each with imgT stationary and W_ia moving, plus a
    one-hot fold matmul adding the per-batch q-projection row.
  - tanh on ScalarE (psum -> bf16 SBUF); logits via one DVE
    tensor_tensor_reduce against a partition-broadcast Wp.
  - Logit columns [128, 25] are PE-transposed then reshaped to [16, 196]
    by a single SBUF->SBUF DMA; softmax is a 3-op sequence on 16 lanes.
  - vI via diag-masked piT stationaries accumulating all 16 batches into
    one PSUM [16, 512] pair; img natural layout is re-streamed from DRAM.
  - u1/u2 transposed once into u1T/u2T [128d, 32b] bf16 for the q-proj of
    block 2 and the final FC.
  - The two 16-batch halves are interleaved so softmax/DVE phases of one
    half hide under the other half's projection matmuls.
"""

import os
import sys

import numpy as np

if "/opt/trn_rl_repo" not in sys.path:
    sys.path.insert(0, "/opt/trn_rl_repo")

B_FULL = 256
N_CORES = 8
B = B_FULL // N_CORES  # 32
BH = 16  # half-batch
S = 196
D = 1024
A = 512
O = 3000
SH = BH * S  # 3136 flat s-cols per half
DC = D // 128  # 8
OC = 6
ON = O // OC  # 500
# flat s-chunks per half: 24 x 128 + 1 x 64
CHUNKS = [(j * 128, 128) for j in range(24)] + [(3072, 64)]
# xpose-DMA windows (s-cols) per half
WINDOWS = [(0, 1024), (1024, 1024), (2048, 1024), (3072, 64)]

_nc_cache = None


def _build_nc():
    import concourse.bacc as bacc
    import concourse.tile as tile
    from concourse import mybir
    import bass_rust  # noqa: F401
    import concourse.bass as bass

    f32 = mybir.dt.float32
    bf16 = mybir.dt.bfloat16
    Tanh = mybir.ActivationFunctionType.Tanh
    Exp = mybir.ActivationFunctionType.Exp
    mult = mybir.AluOpType.mult
    add = mybir.AluOpType.add

    nc = bacc.Bacc("TRN2", target_bir_lowering=False)

    img_h = nc.dram_tensor("img", [B, S, D], bf16, kind="ExternalInput")
    ques_h = nc.dram_tensor("ques", [B, D], f32, kind="ExternalInput")
    wia1_h = nc.dram_tensor("W_ia1", [D, A], bf16, kind="ExternalInput")
    wqa1_h = nc.dram_tensor("W_qa1", [D, A], bf16, kind="ExternalInput")
    bqa1_h = nc.dram_tensor("b_qa1", [A], f32, kind="ExternalInput")
    wp1_h = nc.dram_tensor("Wp1", [A], bf16, kind="ExternalInput")
    wia2_h = nc.dram_tensor("W_ia2", [D, A], bf16, kind="ExternalInput")
    wqa2_h = nc.dram_tensor("W_qa2", [D, A], bf16, kind="ExternalInput")
    bqa2_h = nc.dram_tensor("b_qa2", [A], f32, kind="ExternalInput")
    wp2_h = nc.dram_tensor("Wp2", [A], bf16, kind="ExternalInput")
    wfc_h = nc.dram_tensor("W_fc", [D, O], bf16, kind="ExternalInput")
    bfc_h = nc.dram_tensor("b_fc", [O], f32, kind="ExternalInput")
    sel_h = nc.dram_tensor("SEL", [BH, SH], bf16, kind="ExternalInput")
    identb_h = nc.dram_tensor("IDENTB", [128, 128], bf16, kind="ExternalInput")
    identf_h = nc.dram_tensor("IDENTF", [128, 128], f32, kind="ExternalInput")
    score_h = nc.dram_tensor("score", [B, O], f32, kind="ExternalOutput")
    lcscr_h = nc.dram_tensor("LCSCR", [4, 3200], f32, kind="Internal")

    def bcast_ap(h, n_part, off=0, n=None):
        ap = h[off : off + n] if n is not None else h[:]
        return bass.AP(tensor=ap.tensor, offset=ap.offset, ap=[[0, n_part]] + ap.ap)

    def diag_ap(t_ap, npart, nb):
        # t_ap: tile AP [128, nb, nb]; view [npart, nb] hitting [p, i, i]
        pstride = t_ap.ap[0][0]
        return bass.AP(
            tensor=t_ap.tensor, offset=t_ap.offset, ap=[[pstride, npart], [nb + 1, nb]]
        )

    with tile.TileContext(nc) as tc:
        with (
            tc.tile_pool(name="const", bufs=1) as const,
            tc.tile_pool(name="imgt", bufs=1) as imgt_p,
            tc.tile_pool(name="imgn", bufs=6) as imgn_p,
            tc.tile_pool(name="wst", bufs=2) as wst,
            tc.tile_pool(name="ha", bufs=4) as ha_p,
            tc.tile_pool(name="lc", bufs=2) as lc_p,
            tc.tile_pool(name="work", bufs=2) as work,
            tc.tile_pool(name="uh", bufs=2) as uh_p,
            tc.tile_pool(name="psp", bufs=4, space="PSUM") as psp,
            tc.tile_pool(name="pst", bufs=1, space="PSUM") as pst,
        ):
            # ---------------- constants / prologue ----------------
            identb = const.tile([128, 128], bf16)
            nc.sync.dma_start(out=identb, in_=identb_h[:, :])
            identf = const.tile([128, 128], f32)
            nc.sync.dma_start(out=identf, in_=identf_h[:, :])
            sel = const.tile([BH, SH], bf16)
            nc.sync.dma_start(out=sel, in_=sel_h[:, :])
            wia1 = const.tile([128, DC, A], bf16)
            nc.sync.dma_start(out=wia1, in_=wia1_h[:, :].rearrange("(c p) a -> p c a", p=128))
            wia2 = const.tile([128, DC, A], bf16)
            nc.sync.dma_start(out=wia2, in_=wia2_h[:, :].rearrange("(c p) a -> p c a", p=128))
            wp1b = const.tile([128, A], bf16)
            nc.gpsimd.dma_start(out=wp1b, in_=bcast_ap(wp1_h, 128))
            wp2b = const.tile([128, A], bf16)
            nc.gpsimd.dma_start(out=wp2b, in_=bcast_ap(wp2_h, 128))
            bqa1b = const.tile([BH, A], f32)
            nc.gpsimd.dma_start(out=bqa1b, in_=bcast_ap(bqa1_h, BH))
            bqa2b = const.tile([BH, A], f32)
            nc.gpsimd.dma_start(out=bqa2b, in_=bcast_ap(bqa2_h, BH))

            quesA = {}
            for h in range(2):
                qa = const.tile([BH, D], f32, tag=f"quesA{h}")
                nc.sync.dma_start(out=qa, in_=ques_h[h * BH : (h + 1) * BH, :])
                quesA[h] = qa

            # masks for vI: [s-part, b, b] diag tiles, memset once
            mask0 = const.tile([128, BH, BH], bf16)
            nc.vector.memset(mask0[:, :, :].bitcast(f32), 0.0)
            mask1 = const.tile([128, BH, BH], bf16)
            nc.vector.memset(mask1[:, :, :].bitcast(f32), 0.0)
            masks = [mask0, mask1]

            u1T = const.tile([128, DC, B], bf16)
            u2T = const.tile([128, DC, B], bf16)

            # quesT + QP1 per half
            quesbf = {}
            quesT = const.tile([128, DC, B], bf16)
            for h in range(2):
                qb = work.tile([BH, D], bf16, tag="quesbf")
                nc.vector.tensor_copy(qb, quesA[h])
                quesbf[h] = qb
                for c in range(DC):
                    pt = pst.tile([128, BH], bf16, tag="trb")
                    nc.tensor.transpose(
                        pt, qb[:, c * 128 : (c + 1) * 128], identb[0:BH, 0:BH]
                    )
                    nc.vector.tensor_copy(quesT[:, c, h * BH : (h + 1) * BH], pt)
            QP1 = {}
            for h in range(2):
                qp_ps = psp.tile([BH, A], f32, tag="pp")
                for c in range(DC):
                    wq = wst.tile([128, A], bf16, tag="ws")
                    nc.sync.dma_start(out=wq, in_=wqa1_h[c * 128 : (c + 1) * 128, :])
                    nc.tensor.matmul(
                        qp_ps,
                        quesT[:, c, h * BH : (h + 1) * BH],
                        wq,
                        start=(c == 0),
                        stop=(c == DC - 1),
                    )
                qp = const.tile([BH, A], bf16, tag=f"QP1{h}")
                nc.vector.tensor_add(qp, qp_ps, bqa1b)
                QP1[h] = qp

            # ---------------- imgT via DMA-XBAR transpose ----------------
            imgT = {}

            def load_imgT(h):
                flat = img_h[h * BH : (h + 1) * BH, :, :].rearrange("b s d -> (b s) d")
                for w, (w0, wl) in enumerate(WINDOWS):
                    t = imgt_p.tile([128, DC, wl], bf16, tag=f"imgt_{w}")
                    imgT[(h, w)] = t
                    for c in range(DC):
                        nc.sync.dma_start(
                            out=t[:, c, :],
                            in_=flat[w0 : w0 + wl, c * 128 : (c + 1) * 128],
                            transpose=True,
                        )

            def imgT_chunk(h, s0, sl):
                w = s0 // 1024
                off = s0 - w * 1024
                return imgT[(h, w)], off

            # ---------------- phase helpers ----------------
            Lc = {}

            def proj(h, blk):
                """Projection for half h, block blk -> logits columns Lc[(h,blk)]."""
                wia = wia1 if blk == 0 else wia2
                wpb = wp1b if blk == 0 else wp2b
                qp = QP1[h] if blk == 0 else QP2[h]
                KPROJ = int(os.environ.get("KPROJ", "3"))
                lc = lc_p.tile([128, len(CHUNKS)], f32, tag=f"lc{h}{blk}")
                Lc[(h, blk)] = lc
                if KPROJ >= 3:
                    pass
                else:
                    nc.vector.memset(lc[:, :], 0.0)
                for j, (s0, sl) in enumerate(CHUNKS):
                    if KPROJ < 1:
                        break
                    t, off = imgT_chunk(h, s0, sl)
                    pp = psp.tile([128, A], f32, tag="pp")
                    for c in range(DC):
                        nc.tensor.matmul(
                            pp[0:sl, :],
                            t[:, c, off : off + sl],
                            wia[:, c, :],
                            start=(c == 0),
                            stop=False,
                        )
                    nc.tensor.matmul(
                        pp[0:sl, :], sel[:, s0 : s0 + sl], qp, start=False, stop=True
                    )
                    if KPROJ < 2:
                        continue
                    ha = ha_p.tile([128, A], bf16, tag="ha")
                    nc.scalar.activation(ha[0:sl], pp[0:sl], Tanh)
                    if KPROJ < 3:
                        continue
                    tt = ha_p.tile([128, A], bf16, tag="tt")
                    nc.vector.tensor_mul(tt[0:sl], ha[0:sl], wpb[0:sl])
                    nc.vector.tensor_reduce(
                        lc[0:sl, j : j + 1],
                        tt[0:sl],
                        axis=mybir.AxisListType.X,
                        op=add,
                    )

            def softmax(h, blk):
                """Lc[(h,blk)] -> PI [16, 196] fp32 -> diag masks (bf16)."""
                lc = Lc[(h, blk)]
                pt = pst.tile([len(CHUNKS), 128], f32, tag="trw")
                nc.tensor.transpose(pt, lc, identf)
                lcT = work.tile([len(CHUNKS), 128], f32, tag="lcT")
                nc.vector.tensor_copy(lcT, pt)
                idx = h * 2 + blk
                nc.sync.dma_start(out=lcscr_h[idx, 0:3200], in_=lcT[:, :])
                LT = work.tile([BH, S], f32, tag="LT")
                nc.sync.dma_start(
                    out=LT,
                    in_=lcscr_h[idx, 0:SH].rearrange("(b s) -> b s", b=BH),
                )
                E = work.tile([BH, S], f32, tag="E")
                Z = work.tile([BH, 1], f32, tag="Z")
                nc.scalar.activation(E, LT, Exp, accum_out=Z)
                R = work.tile([BH, 1], f32, tag="R")
                nc.vector.reciprocal(R, Z)
                PI = work.tile([BH, S], f32, tag="PI")
                nc.vector.tensor_scalar_mul(PI, E, R)
                pa = pst.tile([128, BH], f32, tag="tr")
                nc.tensor.transpose(pa, PI[:, 0:128], identf[0:BH, 0:BH])
                nc.vector.tensor_copy(diag_ap(masks[0][:, :, :], 128, BH), pa)
                pb = pst.tile([128, BH], f32, tag="tr")
                nc.tensor.transpose(pb[0:68, :], PI[:, 128:196], identf[0:BH, 0:BH])
                nc.vector.tensor_copy(diag_ap(masks[1][:, :, :], 68, BH), pb[0:68, :])

            def vI_u(h, blk):
                """vI psum pair; u = vI + (ques | u1); returns u tile [16, 1024] f32."""
                vp0 = psp.tile([BH, A], f32, tag="pp")
                vp1 = psp.tile([BH, A], f32, tag="pp")
                vps = [vp0, vp1]
                k = 0
                for b in range(BH):
                    inb = imgN[(h, blk, b)]
                    for si, (p0, pl) in enumerate(((0, 128), (0, 68))):
                        for dh in range(2):
                            nc.tensor.matmul(
                                vps[dh],
                                masks[si][0:pl, b, :],
                                inb[0:pl, si, dh * A : (dh + 1) * A],
                                start=(k == 0),
                                stop=(k == 2 * BH - 1),
                            )
                        k += 1
                other = quesA[h] if blk == 0 else U1[h]
                u = uh_p.tile([BH, D], f32, tag=f"u{blk}_{h}")
                for dh in range(2):
                    nc.vector.tensor_add(
                        u[:, dh * A : (dh + 1) * A], vps[dh], other[:, dh * A : (dh + 1) * A]
                    )
                return u

            def u_transpose(u, uT, h):
                for c in range(DC):
                    pt = pst.tile([128, BH], f32, tag="tr")
                    nc.tensor.transpose(pt, u[:, c * 128 : (c + 1) * 128], identf[0:BH, 0:BH])
                    nc.vector.tensor_copy(uT[:, c, h * BH : (h + 1) * BH], pt)

            def qp2(h):
                qp_ps = psp.tile([BH, A], f32, tag="pp")
                for c in range(DC):
                    wq = wst.tile([128, A], bf16, tag="ws")
                    nc.sync.dma_start(out=wq, in_=wqa2_h[c * 128 : (c + 1) * 128, :])
                    nc.tensor.matmul(
                        qp_ps,
                        u1T[:, c, h * BH : (h + 1) * BH],
                        wq,
                        start=(c == 0),
                        stop=(c == DC - 1),
                    )
                qp = const.tile([BH, A], bf16, tag=f"QP2{h}")
                nc.vector.tensor_add(qp, qp_ps, bqa2b)
                QP2[h] = qp

            def load_imgN(h, blk):
                for b in range(BH):
                    gb = h * BH + b
                    inb = imgn_p.tile([128, 2, D], bf16, tag="imgn")
                    imgN[(h, blk, b)] = inb
                    nc.scalar.dma_start(
                        out=inb[:, 0, :],
                        in_=img_h[gb : gb + 1, 0:128, :].rearrange("o s d -> (o s) d"),
                    )
                    nc.scalar.dma_start(
                        out=inb[0:68, 1, :],
                        in_=img_h[gb : gb + 1, 128:196, :].rearrange("o s d -> (o s) d"),
                    )

            def fc():
                for n in range(OC):
                    bfS = work.tile([BH, ON], f32, tag="bfS")
                    nc.gpsimd.dma_start(out=bfS, in_=bcast_ap(bfc_h, BH, n * ON, ON))
                    wf = wst.tile([128, DC, ON], bf16, tag="wf")
                    nc.sync.dma_start(
                        out=wf,
                        in_=wfc_h[:, :].rearrange("(c p) o -> p c o", p=128)[
                            :, :, n * ON : (n + 1) * ON
                        ],
                    )
                    for h in range(2):
                        fp = psp.tile([BH, ON], f32, tag="pp")
                        for c in range(DC):
                            nc.tensor.matmul(
                                fp,
                                u2T[:, c, h * BH : (h + 1) * BH],
                                wf[:, c, :],
                                start=(c == 0),
                                stop=(c == DC - 1),
                            )
                        sc = work.tile([BH, ON], f32, tag="sc")
                        nc.vector.tensor_add(sc, fp, bfS)
                        nc.sync.dma_start(
                            out=score_h[h * BH : (h + 1) * BH, n * ON : (n + 1) * ON],
                            in_=sc,
                        )

            # ---------------- main schedule: sequential halves ----------------
            imgN = {}
            U1 = {}
            QP2 = {}
            WFC = {}

            STAGE = int(os.environ.get("KSTAGE", "99"))
            for h in range(2):
                if STAGE < 1:
                    break
                load_imgT(h)
                load_imgN(h, 0)
                proj(h, 0)
                if STAGE < 2:
                    continue
                softmax(h, 0)
                if STAGE < 3:
                    continue
                U1[h] = vI_u(h, 0)
                u_transpose(U1[h], u1T, h)
                qp2(h)
                if STAGE < 4:
                    continue
                load_imgN(h, 1)
                proj(h, 1)
                softmax(h, 1)
                if STAGE < 5:
                    continue
                u2h = vI_u(h, 1)
                u_transpose(u2h, u2T, h)

            if STAGE >= 6:
                fc()

    nc.compile()
    return nc


def _get_nc():
    global _nc_cache
    if _nc_cache is None:
        _nc_cache = _build_nc()
    return _nc_cache


def _to_bf16(x):
    import ml_dtypes

    x = np.asarray(x)
    if x.nbytes >= 1 << 22:
        # big tensors: multithreaded conversion via jax CPU
        import jax

        cpu = jax.devices("cpu")[0]
        with jax.default_device(cpu):
            y = jax.jit(
                lambda v: v.astype("bfloat16"), backend="cpu"
            )(x)
            return np.asarray(y)
    return x.astype(ml_dtypes.bfloat16)


def _make_in_maps(inputs):
    import ml_dtypes

    bf = ml_dtypes.bfloat16
    ident = np.eye(128)
    selmat = np.zeros((BH, SH), np.float32)
    for b in range(BH):
        selmat[b, b * S : (b + 1) * S] = 1.0
    img_bf = _to_bf16(inputs["img_feat"])
    shared = {
        "W_ia1": _to_bf16(inputs["W_ia1"]),
        "W_qa1": _to_bf16(inputs["W_qa1"]),
        "b_qa1": np.ascontiguousarray(inputs["b_qa1"], np.float32),
        "Wp1": _to_bf16(inputs["Wp1"]),
        "W_ia2": _to_bf16(inputs["W_ia2"]),
        "W_qa2": _to_bf16(inputs["W_qa2"]),
        "b_qa2": np.ascontiguousarray(inputs["b_qa2"], np.float32),
        "Wp2": _to_bf16(inputs["Wp2"]),
        "W_fc": _to_bf16(inputs["W_fc"]),
        "b_fc": np.ascontiguousarray(inputs["b_fc"], np.float32),
        "SEL": selmat.astype(bf),
        "IDENTB": ident.astype(bf),
        "IDENTF": ident.astype(np.float32),
    }
    in_maps = []
    for c in range(N_CORES):
        sl = slice(c * B, (c + 1) * B)
        m = dict(shared)
        m["img"] = img_bf[sl]
        m["ques"] = np.ascontiguousarray(inputs["ques_feat"][sl], np.float32)
        in_maps.append(m)
    return in_maps


def kernel_run(inputs, trace=False):
    from concourse.bass_utils import run_bass_kernel_spmd

    nc = _get_nc()
    in_maps = _make_in_maps(inputs)
    res = run_bass_kernel_spmd(nc, in_maps, core_ids=list(range(N_CORES)), trace=trace)
    out = np.concatenate([r["score"] for r in res.results], axis=0)
    return out, res


def kernel(**inputs):
    out, _ = kernel_run(inputs)
    return out
